# revision 11
# baseline (speedup 1.0000x reference)
"""
MoE-routing kernel for Trainium2 (8 NeuronCores, SPMD via bass).

Computation (matches the reference):
  attended[b, c] = sum_hw(mn[b, hw] * feat[b, c, hw]),  mn = (m+1e-10)/sum(m+1e-10)
  out[b, a]      = attended[b, :] @ W[inst[b], a, :] + bias[inst[b], a]

Strategy: channel-sharded over 8 cores (CS = 2048/8 = 256 channels each);
host sums the 8 partial [B, A] outputs and adds the bias.  Samples are
sorted by expert on the host so each expert's samples form a contiguous
range.  The mask multiply is folded into the host-side fp16 cast of feat
(fm = feat * mn), so on device the pooling is a pure free-axis sum that
runs on the DVE — the PE does only the expert GEMM.

Per core, per expert group (<=128 samples of one expert):
  1. fm tiles [128c, gsz, 196hw] (2 channel k-tiles) stream on the SP
     HWDGE queue; DVE tensor_reduce sums hw -> att32 [128, KT, gsz];
     Act casts to fp16 att16 (the GEMM stationary).
  2. W[e] streams as [128, KT, A] on the Activation HWDGE queue; per
     512-answer chunk, KT fp16 matmuls accumulate in PSUM; Act evicts
     to an SBUF row tile which DMAs out once per group via SWDGE.
All three DMA queues run concurrently; the kernel is HBM-bandwidth
bound (~52 MB/core) and every engine stage trails the stream it
consumes at per-group granularity.
"""

import sys

if "/opt/trn_rl_repo" not in sys.path:
    sys.path.insert(0, "/opt/trn_rl_repo")

import numpy as np

import concourse.bass as bass
import concourse.mybir as mybir
import concourse.tile as tile
from concourse import bacc
from concourse import bass_utils

# Problem constants (hardcoded; kernel.py must be self-contained)
B = 256          # batch
C = 2048         # channels
HW = 196         # spatial positions (14*14)
E = 16           # experts
A = 3000         # answers
NCORES = 8
CS = C // NCORES  # channel shard per core = 256
P = 128
KT = CS // P      # channel k-tiles per core = 2
CHUNKS = [(c0, min(512, A - c0)) for c0 in range(0, A, 512)]

F32 = mybir.dt.float32
F16 = mybir.dt.float16
AXIS_X = mybir.AxisListType.X
ADD = mybir.AluOpType.add


def _make_groups(counts):
    """[(gstart_in_sorted_order, gsz, expert)] with gsz <= 128."""
    groups = []
    start = 0
    for e in range(E):
        n = int(counts[e])
        g0 = start
        while n > 0:
            gsz = min(n, P)
            groups.append((g0, gsz, e))
            g0 += gsz
            n -= gsz
        start += int(counts[e])
    return groups


def build_program(groups, loop_n=1, do_reduce=True, do_mm=True, do_out=True,
                  fm_bufs=4, pool_mode="split"):
    """Build + compile the per-core Bass program (identical on all cores).

    do_reduce/do_mm/do_out strip compute stages for sim ablations; the
    correctness path always uses the defaults.
    pool_mode: 'fused' = one DMA + one fp16 reduce per group;
               'split' = one DMA + per-kt reduces;
               'f32'   = one DMA + fp32 reduce + Act cast."""
    nc = bacc.Bacc("TRN2", target_bir_lowering=False, debug=False,
                   num_devices=NCORES)

    fm_d = nc.dram_tensor("fm", [P, B, KT, HW], F16, kind="ExternalInput").ap()
    wt_d = nc.dram_tensor("wt", [E, P, KT, A], F16, kind="ExternalInput").ap()
    part_d = nc.dram_tensor("part", [B, A], F16, kind="ExternalOutput").ap()

    import contextlib
    with tile.TileContext(nc) as tc:
        loop_ctx = tc.For_i(0, loop_n, 1) if loop_n > 1 else contextlib.nullcontext()
        with (
            loop_ctx,
            tc.tile_pool(name="persist", bufs=1) as pp,
            tc.tile_pool(name="fm", bufs=fm_bufs) as fp,
            tc.tile_pool(name="a32", bufs=3) as ap32,
            tc.tile_pool(name="wt", bufs=3) as wtp,
            tc.tile_pool(name="outs", bufs=2) as op,
            tc.tile_pool(name="ps_mm", bufs=4, space="PSUM") as pmm,
        ):
            for gi, (g0, gsz, e) in enumerate(groups):
                # --- pooling: fm -> att16 (one DMA, one DVE reduce) ---
                att16 = pp.tile([P, gsz, KT], F16, tag=f"a16_{gi}")
                ft = fp.tile([P, gsz, KT, HW], F16, tag="fm")
                nc.sync.dma_start(ft, fm_d[:, g0:g0 + gsz, :, :])
                if not do_reduce:
                    nc.vector.memset(att16.bitcast(F32), 0.0)
                elif pool_mode == "fused":
                    with nc.allow_low_precision("DVE reduce accumulates fp32"):
                        nc.vector.tensor_reduce(att16, ft, axis=AXIS_X, op=ADD)
                elif pool_mode == "split":
                    with nc.allow_low_precision("DVE reduce accumulates fp32"):
                        for t in range(KT):
                            nc.vector.tensor_reduce(
                                att16[:, :, t], ft[:, :, t, :],
                                axis=AXIS_X, op=ADD)
                elif pool_mode == "f32":
                    att32 = ap32.tile([P, gsz, KT], F32, tag="a32")
                    nc.vector.tensor_reduce(att32, ft, axis=AXIS_X, op=ADD)
                    nc.scalar.copy(att16, att32)

                # --- expert GEMM: att16^T @ W[e] ---
                wt = wtp.tile([P, KT, A], F16, tag="wt")
                nc.scalar.dma_start(wt, wt_d[e])
                ot = op.tile([P, A], F16, tag="out")
                for (c0, cw) in CHUNKS:
                    if not do_mm:
                        continue
                    ps = pmm.tile([P, 512], F32, name="ps")
                    for t in range(KT):
                        nc.tensor.matmul(
                            ps[:gsz, :cw],
                            lhsT=att16[:, :, t],
                            rhs=wt[:, t, c0:c0 + cw],
                            start=(t == 0), stop=(t == KT - 1))
                    nc.scalar.copy(ot[:gsz, c0:c0 + cw], ps[:gsz, :cw])
                if not do_mm:
                    nc.vector.memset(ot[:gsz].bitcast(F32), 0.0)
                if do_out:
                    nc.gpsimd.dma_start(part_d[g0:g0 + gsz, :], ot[:gsz])

    nc.compile()
    return nc


_PROGRAM_CACHE = {}


def _get_program(groups):
    key = tuple(groups)
    if key not in _PROGRAM_CACHE:
        _PROGRAM_CACHE[key] = build_program(groups)
    return _PROGRAM_CACHE[key]


def make_in_maps(mask, features, W, b, inst):
    """Host-side routing + sharding.  Returns (in_maps, perm, groups)."""
    inst_np = np.asarray(inst).astype(np.int64)
    perm = np.argsort(inst_np, kind="stable")
    counts = np.bincount(inst_np, minlength=E)
    groups = _make_groups(counts)

    m = np.asarray(mask, np.float64).reshape(B, HW) + 1e-10
    mn = (m / m.sum(1, keepdims=True)).astype(np.float32)[perm]

    feat = np.asarray(features, np.float32).reshape(B, C, HW)[perm]
    # fold the mask into the fp16 cast: fm[s, c, hw] = feat * mn
    fm16 = (feat * mn[:, None, :]).astype(np.float16)
    Wf = np.asarray(W, np.float32)

    in_maps = []
    for k in range(NCORES):
        sl = slice(k * CS, (k + 1) * CS)
        # fm_k[p, s, t, hw] = fm[s, k*CS + t*128 + p, hw]
        fm_k = np.ascontiguousarray(
            fm16[:, sl].reshape(B, KT, P, HW).transpose(2, 0, 1, 3))
        # wt_k[e, p, t, a] = W[e, a, k*CS + t*128 + p]
        wt_k = np.ascontiguousarray(
            Wf[:, :, sl].transpose(0, 2, 1).reshape(E, KT, P, A)
            .transpose(0, 2, 1, 3)).astype(np.float16)
        in_maps.append({
            "fm": fm_k,
            "wt": wt_k,
        })
    return in_maps, perm, groups


def postprocess(results, perm, b, inst):
    part = np.zeros((B, A), np.float32)
    for r in results:
        part += np.asarray(r["part"], np.float32)
    out = np.empty((B, A), np.float32)
    out[perm] = part
    out += np.asarray(b, np.float32)[np.asarray(inst).astype(np.int64)]
    return out


def kernel(mask, features, W, b, inst):
    in_maps, perm, groups = make_in_maps(mask, features, W, b, inst)
    nc = _get_program(groups)
    res = bass_utils.run_bass_kernel_spmd(nc, in_maps, core_ids=list(range(NCORES)))
    return postprocess(res.results, perm, b, inst)
